# revision 7
# baseline (speedup 1.0000x reference)
"""Trainium2 Bass kernel for the counting-criterion loss.

Computes, for output/density_map of shape [32, 1, 512, 512] and bboxes [32, 3, 4]:
  dmap_loss  = sum((output - density_map)^2) / num_objects
  count_loss = mean_b((sum(output_b) - sum(density_map_b))^2)
  min_count  = sum_boxes(relu(1 - box_sum))   with box sums over [y1:y2, x1:x2)

Strategy: data-parallel over the batch -- core i handles images [4i, 4i+4).
Inputs are cast to bf16 on the host (rel-err impact ~3.5e-3, well inside the
2e-2 gate), halving HBM traffic vs f32; the DMA stream is the roofline.
Per core:
  - DVE tensor_tensor (2x mode): diff = o - d for the big pieces
  - per-image sum(diff) via PE: ones-vector matmuls over 128-col blocks of
    the diff tiles accumulate column sums into a psum tile (one col/image)
  - ACT Square activation with accum_out: per-partition sum(diff^2)
  - box row-sums on PE: psum[x, (cx,j)] += O_block^T @ rowmask over y-chunks;
    col-mask multiply and final tiny reductions happen on the host
  - the last image's d arrives as a tapered sequence; the final pieces use
    scalar_tensor_tensor (diff+accum) and DVE squares into a separate acct
    tile, so the post-stream critical path is a short single-engine chain
Stream order feeds ACT continuously: img0 halves, img3 main, img1, img2,
then the img3 taper. Output: a main acc DMA (ACT queue) that depends only on
early work, and a tiny acct DMA at the end.
"""

import numpy as np
import ml_dtypes
from contextlib import ExitStack

import concourse.bass as bass
import concourse.mybir as mybir
import concourse.tile as tile
from concourse import bacc
from concourse.bass_utils import run_bass_kernel_spmd

N_CORES = 8
B, H, W = 32, 512, 512
NIMG = B // N_CORES  # images per core
P = 128              # SBUF partitions
NCH = H // P         # row chunks per image (and col chunks: W//P)
NB = 3               # boxes per image
F32 = mybir.dt.float32
BF16 = mybir.dt.bfloat16

# acc layout (f32):
#   sq cols: img0, img3 h0, img3 c2, img3 c3pre, img1, img2 h0, img2 h1a,
#            img2 h1b                                        -> 8
#   psum_sum copy: per-image PE sum(diff) partials           -> 4
#   box psum copies: 12 per image                            -> 48
NSQ = 8
ACC_SUM = NSQ
ACC_BOX = NSQ + NIMG
NACC = ACC_BOX + NIMG * NCH * NB  # 60
# acct layout: c3pre stt, c3f stt, c3l stt, c3pre sq, c3f sq, c3l sq
NACCT = 6

_PROG = None


def _build_program():
    nc = bacc.Bacc(
        "TRN2",
        target_bir_lowering=False,
        debug=False,
        num_devices=N_CORES,
    )
    o_d = nc.dram_tensor("o", [NIMG, H, W], BF16, kind="ExternalInput").ap()
    d_d = nc.dram_tensor("d", [NIMG, H, W], BF16, kind="ExternalInput").ap()
    # row masks, host-packed as [p, (img, cy, j)]
    rm_d = nc.dram_tensor("rm", [P, NIMG * NCH * NB], BF16, kind="ExternalInput").ap()
    acc_d = nc.dram_tensor("acc", [P, NACC], F32, kind="ExternalOutput").ap()
    acct_d = nc.dram_tensor("acct", [P, NACCT], F32, kind="ExternalOutput").ap()

    # DRAM views: image rows split as y = c*128 + p  ->  [img, p, c, x]
    o_r = o_d.rearrange("n (c p) x -> n p c x", p=P)
    d_r = d_d.rearrange("n (c p) x -> n p c x", p=P)

    with tile.TileContext(nc) as tc, ExitStack() as ctx:
        io_pool = ctx.enter_context(tc.tile_pool(name="io", bufs=1))
        work_pool = ctx.enter_context(tc.tile_pool(name="work", bufs=1))
        psum_pool = ctx.enter_context(tc.tile_pool(name="psum", bufs=1, space="PSUM"))
        acc_pool = ctx.enter_context(tc.tile_pool(name="acc", bufs=1))

        acc = acc_pool.tile([P, NACC], F32)
        acct = acc_pool.tile([P, NACCT], F32)
        ones_t = acc_pool.tile([P, 1], BF16)
        warm = acc_pool.tile([P, 1], F32)
        nc.vector.memset(acc[:], 0.0)
        nc.vector.memset(acct[:], 0.0)
        nc.vector.memset(ones_t[:], 1.0)
        nc.vector.memset(warm[:], 0.0)
        # force the Square act-table load early so it hides under the first DMA
        nc.scalar.activation(warm[:], warm[:], mybir.ActivationFunctionType.Square)

        o_tiles = [
            io_pool.tile([P, NCH, W], BF16, tag=f"o{i}", name=f"o{i}")
            for i in range(NIMG)
        ]
        d_tiles = [
            io_pool.tile([P, NCH, W], BF16, tag=f"d{i}", name=f"d{i}")
            for i in range(NIMG)
        ]
        diff_tiles = [
            work_pool.tile([P, NCH, W], BF16, tag=f"f{i}", name=f"f{i}")
            for i in range(NIMG)
        ]
        sq_tiles = [
            work_pool.tile([P, NCH, W], BF16, tag=f"s{i}", name=f"s{i}")
            for i in range(NIMG)
        ]
        rm_t = io_pool.tile([P, NIMG, NCH * NB], BF16, tag="rm")
        psum_sum = psum_pool.tile([P, NIMG], F32, tag="psum_sum", name="psum_sum")

        def tt_sub(img, sl):
            nc.vector.tensor_tensor(
                out=diff_tiles[img][sl],
                in0=o_tiles[img][sl],
                in1=d_tiles[img][sl],
                op=mybir.AluOpType.subtract,
            )

        def pe_sum(img, blocks, start, stop):
            """Column sums of diff 128-col blocks into psum_sum[:, img]."""
            for bi, (c, x0) in enumerate(blocks):
                nc.tensor.matmul(
                    psum_sum[:, img : img + 1],
                    lhsT=diff_tiles[img][:, c, x0 : x0 + P],
                    rhs=ones_t[:],
                    start=start and bi == 0,
                    stop=stop and bi == len(blocks) - 1,
                )

        def stt_sub(img, sl, col):
            nc.vector.scalar_tensor_tensor(
                out=diff_tiles[img][sl],
                in0=o_tiles[img][sl],
                scalar=0.0,
                in1=d_tiles[img][sl],
                op0=mybir.AluOpType.bypass,
                op1=mybir.AluOpType.subtract,
                accum_out=acct[:, col : col + 1],
            )

        def sq_act(img, sl, col, acct_out=False):
            tgt = acct if acct_out else acc
            nc.scalar.activation(
                sq_tiles[img][sl],
                diff_tiles[img][sl],
                mybir.ActivationFunctionType.Square,
                accum_out=tgt[:, col : col + 1],
            )

        def sq_dve(img, sl, col):
            nc.vector.scalar_tensor_tensor(
                out=sq_tiles[img][sl],
                in0=diff_tiles[img][sl],
                scalar=0.0,
                in1=diff_tiles[img][sl],
                op0=mybir.AluOpType.bypass,
                op1=mybir.AluOpType.mult,
                accum_out=acct[:, col : col + 1],
            )

        def boxes(img):
            ps = psum_pool.tile([P, NCH * NB], F32, tag=f"ps{img}", name=f"ps{img}")
            for cx in range(NCH):
                for cy in range(NCH):
                    nc.tensor.matmul(
                        ps[:, cx * NB : (cx + 1) * NB],
                        lhsT=o_tiles[img][:, cy, cx * P : (cx + 1) * P],
                        rhs=rm_t[:, img, cy * NB : (cy + 1) * NB],
                        start=(cy == 0),
                        stop=(cy == NCH - 1),
                    )
            col0 = ACC_BOX + img * NCH * NB
            nc.vector.tensor_copy(acc[:, col0 : col0 + NCH * NB], ps[:])

        HC = NCH // 2
        lo = np.s_[:, 0:HC]
        hi = np.s_[:, HC:NCH]
        blocks_lo = [(c, x0) for c in range(HC) for x0 in range(0, W, P)]
        blocks_hi = [(c, x0) for c in range(HC, NCH) for x0 in range(0, W, P)]

        # ---- image 0: halves; square on ACT per image ----------------------
        nc.sync.dma_start(o_tiles[0][lo], o_r[0, :, 0:HC])
        nc.sync.dma_start(d_tiles[0][lo], d_r[0, :, 0:HC])
        nc.sync.dma_start(rm_t[:], rm_d)
        tt_sub(0, lo)
        pe_sum(0, blocks_lo, start=True, stop=False)
        nc.sync.dma_start(o_tiles[0][hi], o_r[0, :, HC:NCH])
        nc.sync.dma_start(d_tiles[0][hi], d_r[0, :, HC:NCH])
        tt_sub(0, hi)
        pe_sum(0, blocks_hi, start=False, stop=True)
        sq_act(0, np.s_[:], 0)
        boxes(0)

        # ---- image 3 main: o full, d h0 + c2 (c3 comes in the taper) -------
        nc.sync.dma_start(o_tiles[3][:], o_r[3])
        nc.sync.dma_start(d_tiles[3][lo], d_r[3, :, 0:HC])
        tt_sub(3, lo)
        pe_sum(3, blocks_lo, start=True, stop=False)
        sq_act(3, lo, 1)
        boxes(3)
        c2 = np.s_[:, 2, 0:W]
        nc.sync.dma_start(d_tiles[3][c2], d_r[3, :, 2, 0:W])
        tt_sub(3, c2)
        pe_sum(3, [(2, x0) for x0 in range(0, W, P)], start=False, stop=True)
        sq_act(3, c2, 2)

        # ---- image 1: full-image pieces ------------------------------------
        nc.sync.dma_start(o_tiles[1][:], o_r[1])
        nc.sync.dma_start(d_tiles[1][:], d_r[1])
        tt_sub(1, np.s_[:])
        pe_sum(1, blocks_lo + blocks_hi, start=True, stop=True)
        sq_act(1, np.s_[:], 4)
        boxes(1)

        # ---- image 2: o full, d as h0 + c2 + c3 ----------------------------
        nc.sync.dma_start(o_tiles[2][:], o_r[2])
        nc.sync.dma_start(d_tiles[2][lo], d_r[2, :, 0:HC])
        tt_sub(2, lo)
        pe_sum(2, blocks_lo, start=True, stop=False)
        sq_act(2, lo, 5)
        boxes(2)
        # copy of pe sums for images 0/1/3 would race img2; single copy below
        nc.sync.dma_start(d_tiles[2][c2], d_r[2, :, 2, 0:W])
        tt_sub(2, c2)
        pe_sum(2, [(2, x0) for x0 in range(0, W, P)], start=False, stop=False)
        sq_act(2, c2, 6)
        c3 = np.s_[:, 3, 0:W]
        nc.sync.dma_start(d_tiles[2][c3], d_r[2, :, 3, 0:W])
        tt_sub(2, c3)
        pe_sum(2, [(3, x0) for x0 in range(0, W, P)], start=False, stop=True)
        sq_act(2, c3, 7)

        # pe sums complete for all images except img3's c3 taper -> copy now
        nc.vector.tensor_copy(acc[:, ACC_SUM : ACC_SUM + NIMG], psum_sum[:])

        # ---- img3 taper: c3 = 256 (stt + ACT sq) + 128 + 128 (stt + DVE sq)
        c3pre = np.s_[:, 3, 0:256]
        nc.sync.dma_start(d_tiles[3][c3pre], d_r[3, :, 3, 0:256])
        stt_sub(3, c3pre, 0)
        sq_act(3, c3pre, 3, acct_out=True)

        # main acc out on the ACT HWDGE queue (independent of the taper tail)
        nc.scalar.dma_start(acc_d, acc[:])

        c3f = np.s_[:, 3, 256:384]
        nc.sync.dma_start(d_tiles[3][c3f], d_r[3, :, 3, 256:384])
        stt_sub(3, c3f, 1)
        c3l = np.s_[:, 3, 384:512]
        nc.sync.dma_start(d_tiles[3][c3l], d_r[3, :, 3, 384:512])
        stt_sub(3, c3l, 2)
        sq_dve(3, c3f, 4)
        sq_dve(3, c3l, 5)
        nc.sync.dma_start(acct_d, acct[:])

    nc.compile()
    return nc


def _get_program():
    global _PROG
    if _PROG is None:
        _PROG = _build_program()
    return _PROG


def _prep_inputs(output, density_map, bboxes):
    o = np.asarray(output, dtype=np.float32).reshape(B, H, W).astype(ml_dtypes.bfloat16)
    dm = (
        np.asarray(density_map, dtype=np.float32)
        .reshape(B, H, W)
        .astype(ml_dtypes.bfloat16)
    )
    bb = np.clip(np.asarray(bboxes).astype(np.int64), 0, W).astype(np.int32)
    x1, y1, x2, y2 = bb[..., 0], bb[..., 1], bb[..., 2], bb[..., 3]
    x2 = np.maximum(x2, x1)
    y2 = np.maximum(y2, y1)

    ar = np.arange(H, dtype=np.int32)
    # rm[b, y, j] = 1 if y1 <= y < y2, packed as [b, p, (cy, j)]
    rm = (
        (ar[None, :, None] >= y1[:, None, :]) & (ar[None, :, None] < y2[:, None, :])
    ).astype(np.float32)
    rm = rm.reshape(B, NCH, P, NB).transpose(0, 2, 1, 3).astype(ml_dtypes.bfloat16)
    # col mask stays on the host: cm[b, x, j]
    cm = (
        (ar[None, :, None] >= x1[:, None, :]) & (ar[None, :, None] < x2[:, None, :])
    ).astype(np.float64)
    return o, dm, rm, cm


def kernel(output, density_map, bboxes, num_objects):
    o, dm, rm, cm = _prep_inputs(output, density_map, bboxes)

    nc = _get_program()
    in_maps = [
        {
            "o": np.ascontiguousarray(o[i * NIMG : (i + 1) * NIMG]),
            "d": np.ascontiguousarray(dm[i * NIMG : (i + 1) * NIMG]),
            # [p, img, cy, j] flattened to [p, img*12]
            "rm": np.ascontiguousarray(
                rm[i * NIMG : (i + 1) * NIMG].transpose(1, 0, 2, 3).reshape(P, -1)
            ),
        }
        for i in range(N_CORES)
    ]
    res = run_bass_kernel_spmd(nc, in_maps, core_ids=list(range(N_CORES)))

    per_img_d = []   # [B] sum(o - d) per image
    sq_total = 0.0
    box_sums = []    # [B, NB]
    for ci, r in enumerate(res.results):
        a = r["acc"].astype(np.float64)
        at = r["acct"].astype(np.float64)
        sq_total += a[:, :NSQ].sum() + at[:, 3:6].sum()
        sums = a[:, ACC_SUM : ACC_SUM + NIMG].sum(axis=0)  # PE sums per image
        taper = at[:, 0:3].sum()  # img3 c3 pieces
        per_img_d.extend([sums[0], sums[1], sums[2], sums[3] + taper])
        # box partials: [128, (img, cx, j)]; full x index = cx * 128 + p
        bp = a[:, ACC_BOX:].reshape(P, NIMG, NCH, NB)
        for k in range(NIMG):
            img = ci * NIMG + k
            bx = bp[:, k].transpose(1, 0, 2).reshape(W, NB)
            box_sums.append((bx * cm[img]).sum(axis=0))

    dmap_loss = sq_total / float(num_objects)
    count_loss = float(np.mean(np.asarray(per_img_d) ** 2))
    min_count = float(np.maximum(0.0, 1.0 - np.asarray(box_sums)).sum())
    return np.array([dmap_loss, count_loss, min_count], dtype=np.float32)
